# revision 28
# baseline (speedup 1.0000x reference)
"""AdaptiveFusion kernel for 8 TRN2 NeuronCores.

Computes, for xs [V=3, N=131072, D=512], alpha_w [512], alpha_b [1]:
    logits = leaky_relu(einsum('vnd,d->vn', xs, alpha_w) + alpha_b, 0.01)
    attn   = softmax(logits, axis=0)           # over the V=3 views
    out    = einsum('vn,vnd->nd', attn, xs)    # [N, D]

Data-parallel over the node axis N: each of the 8 cores handles
N_local = 16384 nodes; alpha_w/alpha_b replicated; no collectives.
xs/alpha_w are bf16 on the wire (halves HBM traffic); softmax and PSUM
accumulation stay fp32; rel err vs the fp32 reference ~3e-3 (gate 2e-2).

Per-core structure (build_nc2): supertiles of npp*128 nodes loaded
v-major ([p, v, m, d]) so every DMA moves 4KB-contiguous runs on both
sides; groups of `grp` supertiles batch the softmax smalls; emission is
software-pipelined (`depth` groups between a group's loads/dots and its
diag/matmul/normalize/store tail) so the in-order engine queues never
convoy on the long per-group dependency chain.

Engine assignment (measured costs, ns per op):
  - dots: DVE scalar_tensor_tensor (x*w, accum_out) [128,512] ~555 each;
    a tunable fraction runs as DVE 2x tensor_mul (~330) + ScalarE
    accumulate-copy (~875) to balance DVE vs ScalarE (dot_asgn).
  - leaky_relu fused as one DVE STT: max(z*0.01, z); exp on ScalarE;
    1/den folded into the attention weights a_v = e_v/den (one DVE op),
    so the PSUM evacuation is a plain cast copy.
  - diag(a_v) builds on ScalarE (activation Copy, scale=a_v[p,1], ~385);
    drain groups build them on DVE (~163) once the dot stream ends.
  - TensorE: psum[:, m*D:(m+1)*D] += diag(a_v) @ x_v at K=M=128, N=512
    bf16; the supertile shares one [128, 2048] PSUM tile (4 banks,
    2 bufs = all 8), evacuated by ONE ScalarE copy per supertile
    (amortizes the ~220-cycle ScalarE SBUF latency, ~1.9us/supertile).
  - stores lag `store_lag` groups so the store backlog keeps the DMA
    engines fed through the compute drain at the end.

Notes from HW probing: GPSIMD is unusable here (~2us fixed dispatch per
op); neuronxcc rejects scalar_tensor_tensor/tensor_reduce on Pool;
DVE 2-input-with-accum ops run 1x (no 2x uop), so the 384 dot STTs
(~213us) are the hard DVE floor. DMA floor for the 67MB/core traffic
is ~187us. Measured exec ~270-285us (run-to-run HBM variance +-5%).
"""

import ml_dtypes
import numpy as np
from contextlib import ExitStack

import concourse.bass as bass
import concourse.tile as tile
from concourse import bacc, mybir
from concourse.bass_utils import run_bass_kernel_spmd

V = 3
N = 131072
D = 512
NCORES = 8
NL = N // NCORES          # 16384 nodes per core
TILE_N = 128              # nodes per tile (partition dim)
NT = NL // TILE_N         # 128 tiles per core
NEG_SLOPE = 0.01

F32 = mybir.dt.float32
BF16 = mybir.dt.bfloat16
AF = mybir.ActivationFunctionType
ALU = mybir.AluOpType


def build_nc(reps: int = 1, nt: int = NT, grp: int = 4, diag_acts: int = 2,
             x_bufs: int = 16, scr_bufs: int = 8, psum_bufs: int = 6,
             out_bufs: int = 8, npp: int = 1, act_reduce: int = 0,
             sm_bufs: int = 4, diag_bufs: int = 12,
             scr_dummy: bool = False) -> bass.Bass:
    nc = bacc.Bacc("TRN2", target_bir_lowering=False, debug=False)
    nl = nt * TILE_N
    # xs/alpha_w are pre-converted to bf16 on the host: halves HBM read
    # traffic and lets the loads use HWDGE (no SWDGE descriptor-gen).
    xs = nc.declare_dram_parameter("xs", [V, nl, D], BF16, isOutput=False)
    # host-prebuilt constants: broadcast alpha_w, alpha_b column, identity
    awb = nc.declare_dram_parameter("w_bcast", [128, D], BF16, isOutput=False)
    abc = nc.declare_dram_parameter("b_colv", [128, 1], F32, isOutput=False)
    idn = nc.declare_dram_parameter("identity", [128, 128], BF16, isOutput=False)
    idn3 = nc.declare_dram_parameter("identity3", [128, V * 128], BF16, isOutput=False)
    out = nc.declare_dram_parameter("out", [nl, D], BF16, isOutput=True)

    GRP = grp  # supertiles per group (batches the small softmax ops)
    NT_ = nt
    # buffer counts are in 128-node units; scale down for bigger supertiles
    x_bufs = max(3, x_bufs // npp)
    out_bufs = max(3, out_bufs // npp)

    with ExitStack() as ctx:
        tc = ctx.enter_context(tile.TileContext(nc))
        const_pool = ctx.enter_context(tc.tile_pool(name="const", bufs=1))
        x_pool = ctx.enter_context(tc.tile_pool(name="x", bufs=x_bufs))
        scr_pool = ctx.enter_context(tc.tile_pool(name="scr", bufs=scr_bufs))
        sm_pool = ctx.enter_context(tc.tile_pool(name="sm", bufs=sm_bufs))
        diag_pool = ctx.enter_context(tc.tile_pool(name="diag", bufs=diag_bufs))
        out_pool = ctx.enter_context(tc.tile_pool(name="outp", bufs=out_bufs))
        psum_pool = ctx.enter_context(tc.tile_pool(name="psum", bufs=psum_bufs, space="PSUM"))

        # ---- constants (host-prebuilt, one HWDGE DMA each) ----
        w_bc = const_pool.tile([128, D], BF16)
        nc.sync.dma_start(w_bc[:, :], awb[:, :])
        b_col = const_pool.tile([128, 1], F32)
        nc.sync.dma_start(b_col[:, :], abc[:, :])
        ident = const_pool.tile([128, 128], BF16)
        nc.sync.dma_start(ident[:, :], idn[:, :])

        # ---- main loop ----
        # supertile = npp*128 nodes (node n0 + p*npp + m on partition p);
        # groups of GRP supertiles batch the small softmax ops.
        NST = NT_ // npp
        assert NST % GRP == 0
        SW = GRP * npp  # softmax columns per group (x V)
        for g in range((NST // GRP) * reps):
            g = g % (NST // GRP)
            n0g = g * GRP * npp * TILE_N
            lgt = sm_pool.tile([128, SW * V], F32)
            x_ts = []
            for j in range(GRP):
                n0 = n0g + j * npp * TILE_N
                x_t = x_pool.tile([128, npp * V * D], BF16)
                x_ts.append(x_t)
                if npp == 1:
                    src = xs[:, n0:n0 + TILE_N, :].rearrange("v n d -> n v d")
                    dst = x_t[:, :].rearrange("p (v d) -> p v d", v=V)
                    nc.sync.dma_start(dst, src)
                else:
                    dst4 = x_t[:, :].rearrange(
                        "p (m v d) -> p m v d", m=npp, v=V)
                    for v in range(V):
                        src = xs[v, n0:n0 + npp * TILE_N, :].rearrange(
                            "(p m) d -> p m d", m=npp)
                        nc.sync.dma_start(dst4[:, :, v, :], src)
                for m in range(npp):
                    for v in range(V):
                        c = (j * npp + m) * V + v
                        xsl = x_t[:, (m * V + v) * D:(m * V + v + 1) * D]
                        scr = scr_pool.tile([128, D], BF16)
                        if v < act_reduce:
                            # DVE multiply; free-axis sum on ScalarE
                            nc.vector.tensor_mul(scr[:, :], xsl, w_bc[:, :])
                            dummy = scr_pool.tile([128, 1], BF16, tag="dummy")
                            nc.scalar.activation(
                                dummy[:, :].broadcast_to((128, D)), scr[:, :],
                                AF.Copy, accum_out=lgt[:, c:c + 1],
                            )
                        else:
                            if scr_dummy:
                                sd = scr_pool.tile([128, 1], BF16, tag="sd")
                                so = sd[:, :].broadcast_to((128, D))
                            else:
                                so = scr[:, :]
                            # out = (x*1.0)*w ; accum_out = sum(out) = <x, w>
                            nc.vector.scalar_tensor_tensor(
                                out=so,
                                in0=xsl,
                                scalar=1.0,
                                in1=w_bc[:, :],
                                op0=ALU.mult,
                                op1=ALU.mult,
                                accum_out=lgt[:, c:c + 1],
                            )

            # batched leaky_relu(dot + b): z = dot+b ; lrl = max(z, 0.01*z)
            z_t = sm_pool.tile([128, SW * V], F32)
            nc.vector.tensor_scalar_add(z_t[:, :], lgt[:, :], b_col[:, :])
            z1_t = sm_pool.tile([128, SW * V], F32)
            nc.vector.tensor_scalar_mul(z1_t[:, :], z_t[:, :], NEG_SLOPE)
            lrl = sm_pool.tile([128, SW * V], F32)
            nc.vector.tensor_max(lrl[:, :], z_t[:, :], z1_t[:, :])

            e_t = sm_pool.tile([128, SW * V], F32)
            nc.scalar.activation(e_t[:, :], lrl[:, :], AF.Exp)
            den = sm_pool.tile([128, SW], F32)
            nc.vector.tensor_reduce(
                den[:, :], e_t[:, :].rearrange("p (s v) -> p s v", v=V),
                axis=mybir.AxisListType.X, op=ALU.add,
            )
            rc = sm_pool.tile([128, SW], F32)
            nc.vector.reciprocal(rc[:, :], den[:, :])

            for j in range(GRP):
                x_t = x_ts[j]
                o_t = out_pool.tile([128, npp * D], BF16)
                for m in range(npp):
                    s = j * npp + m
                    diag = diag_pool.tile([128, V * 128], BF16)
                    # build diag(e_v): split across DVE and ScalarE
                    for v in range(V):
                        sl = diag[:, v * 128:(v + 1) * 128]
                        ev = e_t[:, s * V + v:s * V + v + 1]
                        if v < V - diag_acts:
                            nc.vector.tensor_scalar_mul(sl, ident[:, :], ev)
                        else:
                            nc.scalar.activation(
                                sl, ident[:, :], AF.Copy, bias=0.0, scale=ev,
                            )

                    ps = psum_pool.tile([128, D], F32)
                    for v in range(V):
                        nc.tensor.matmul(
                            ps[:, :],
                            diag[:, v * 128:(v + 1) * 128],
                            x_t[:, (m * V + v) * D:(m * V + v + 1) * D],
                            start=(v == 0),
                            stop=(v == V - 1),
                        )

                    if drain and (m % 2 == 0):
                        nc.vector.tensor_copy(
                            o_t[:, m * D:(m + 1) * D], ps[:, :])
                    else:
                        nc.scalar.activation(
                            o_t[:, m * D:(m + 1) * D], ps[:, :], AF.Copy)
                n0 = n0g + j * npp * TILE_N
                if npp == 1:
                    nc.sync.dma_start(out[n0:n0 + TILE_N, :], o_t[:, :])
                else:
                    dstO = out[n0:n0 + npp * TILE_N, :].rearrange(
                        "(p m) d -> p m d", m=npp)
                    nc.sync.dma_start(
                        dstO, o_t[:, :].rearrange("p (m d) -> p m d", m=npp))

    nc.compile()
    return nc


def build_nc2(nt: int = NT, npp: int = 4, grp: int = 2,
              dot_asgn: str = "ddd,ddd,ddd,ddd",
              diag_asgn: str = "aaa,aaa,aaa,aaa",
              x_bufs: int = 7, scr_bufs: int = 16, psum_bufs: int = 2,
              out_bufs: int = 14, sm_bufs: int = 24, diag_bufs: int = 24,
              smalls: str = "v", store_eng: str = "s",
              depth: int = 2, const_eng: str = "a",
              tail_norm_dve: int = 1, store_lag: int = 5,
              taper: int = 4, front_taper: int = 2) -> bass.Bass:
    """v2: npp-node supertiles (v-major x layout, 4KB-contiguous DMAs both
    sides), per-slot engine assignment spreading the per-node dot products
    and diag builds across DVE ('d'), GPSIMD/Pool ('p'), and ScalarE ('a',
    dot = DVE 2x-mode multiply + ScalarE accumulate-copy), and software-
    pipelined emission: the tail of group g (diag/matmul/normalize/store)
    is emitted `depth` groups after its head (loads/dots/softmax), so the
    in-order engine queues never convoy on the long per-group chain.
    """
    nc = bacc.Bacc("TRN2", target_bir_lowering=False, debug=False)
    nl = nt * TILE_N
    xs = nc.declare_dram_parameter("xs", [V, nl, D], BF16, isOutput=False)
    awb = nc.declare_dram_parameter("w_bcast", [128, D], BF16, isOutput=False)
    abc = nc.declare_dram_parameter("b_colv", [128, 1], F32, isOutput=False)
    idn = nc.declare_dram_parameter("identity", [128, 128], BF16, isOutput=False)
    idn3 = nc.declare_dram_parameter("identity3", [128, V * 128], BF16, isOutput=False)
    out = nc.declare_dram_parameter("out", [nl, D], BF16, isOutput=True)

    dot_pat = [list(s) for s in dot_asgn.split(",")]
    diag_pat = [list(s) for s in diag_asgn.split(",")]
    assert len(dot_pat) == npp and all(len(p) == V for p in dot_pat)
    assert len(diag_pat) == npp and all(len(p) == V for p in diag_pat)

    with ExitStack() as ctx:
        tc = ctx.enter_context(tile.TileContext(nc))
        const_pool = ctx.enter_context(tc.tile_pool(name="const", bufs=1))
        x_pool = ctx.enter_context(tc.tile_pool(name="x", bufs=x_bufs))
        scr_pool = ctx.enter_context(tc.tile_pool(name="scr", bufs=scr_bufs))
        sm_pool = ctx.enter_context(tc.tile_pool(name="sm", bufs=sm_bufs))
        diag_pool = ctx.enter_context(tc.tile_pool(name="diag", bufs=diag_bufs))
        out_pool = ctx.enter_context(tc.tile_pool(name="outp", bufs=out_bufs))
        psum_pool = ctx.enter_context(
            tc.tile_pool(name="psum", bufs=psum_bufs, space="PSUM"))

        ce = nc.scalar if const_eng == "a" else nc.sync
        w_bc = const_pool.tile([128, D], BF16)
        ce.dma_start(w_bc[:, :], awb[:, :])
        b_col = const_pool.tile([128, 1], F32)
        ce.dma_start(b_col[:, :], abc[:, :])
        ident = const_pool.tile([128, 128], BF16)
        ce.dma_start(ident[:, :], idn[:, :])
        ident3 = const_pool.tile([128, V * 128], BF16)
        ce.dma_start(ident3[:, :], idn3[:, :])

        sm_eng = nc.vector if smalls == "v" else nc.gpsimd
        store = {"a": nc.scalar, "s": nc.sync, "p": nc.gpsimd}[store_eng]

        NST = nt // npp

        def head(s0, cnt, drain=False):
            n0g = s0 * npp * TILE_N
            SW = cnt * npp
            lgt = sm_pool.tile([128, SW * V], F32)
            x_ts = []
            for j in range(cnt):
                n0 = n0g + j * npp * TILE_N
                x_t = x_pool.tile([128, V * npp * D], BF16)
                x_ts.append(x_t)
                dst4 = x_t[:, :].rearrange("p (v m d) -> p v m d", v=V, m=npp)
                for v in range(V):
                    src = xs[v, n0:n0 + npp * TILE_N, :].rearrange(
                        "(p m) d -> p m d", m=npp)
                    if s0 < front_taper:
                        # fine-grained first loads: the first dots start
                        # after ~1/npp of the wire time
                        for m in range(npp):
                            nc.sync.dma_start(dst4[:, v, m, :], src[:, m, :])
                    else:
                        nc.sync.dma_start(dst4[:, v, :, :], src)
                for m in range(npp):
                    for v in range(V):
                        c = (j * npp + m) * V + v
                        xsl = x_t[:, (v * npp + m) * D:(v * npp + m + 1) * D]
                        eng = dot_pat[m][v]
                        if drain and eng in ("a", "q"):
                            eng = "d"
                        if eng in ("a", "q"):
                            # product on DVE (2x bf16) or Pool; free-axis
                            # accumulate on ScalarE (accum_out)
                            mul_e = nc.vector if eng == "a" else nc.gpsimd
                            scr = scr_pool.tile([128, D], BF16)
                            mul_e.tensor_mul(scr[:, :], xsl, w_bc[:, :])
                            dummy = scr_pool.tile([128, 1], BF16, tag="dummy")
                            nc.scalar.activation(
                                dummy[:, :].broadcast_to((128, D)), scr[:, :],
                                AF.Copy, accum_out=lgt[:, c:c + 1],
                            )
                        else:
                            scr = scr_pool.tile([128, D], BF16)
                            nc.vector.scalar_tensor_tensor(
                                out=scr[:, :],
                                in0=xsl,
                                scalar=1.0,
                                in1=w_bc[:, :],
                                op0=ALU.mult,
                                op1=ALU.mult,
                                accum_out=lgt[:, c:c + 1],
                            )

            z_t = sm_pool.tile([128, SW * V], F32)
            sm_eng.tensor_scalar_add(z_t[:, :], lgt[:, :], b_col[:, :])
            # lrelu in one fused op: max(z * NEG_SLOPE, z)
            lrl = sm_pool.tile([128, SW * V], F32)
            nc.vector.scalar_tensor_tensor(
                out=lrl[:, :], in0=z_t[:, :], scalar=NEG_SLOPE,
                in1=z_t[:, :], op0=ALU.mult, op1=ALU.max)

            e_t = sm_pool.tile([128, SW * V], F32)
            nc.scalar.activation(e_t[:, :], lrl[:, :], AF.Exp)
            return (s0, cnt, x_ts, e_t)

        def mid(st):
            s0, cnt, x_ts, e_t = st
            SW = cnt * npp
            # post-exp softmax pieces, emitted one group after the Exp was
            # issued so DVE never head-of-line blocks waiting on it
            den = sm_pool.tile([128, SW], F32)
            sm_eng.tensor_reduce(
                den[:, :], e_t[:, :].rearrange("p (s v) -> p s v", v=V),
                axis=mybir.AxisListType.X, op=ALU.add,
            )
            rc = sm_pool.tile([128, SW], F32)
            nc.vector.reciprocal(rc[:, :], den[:, :])
            a_t = sm_pool.tile([128, SW * V], F32)
            rc_bc = rc[:, :].rearrange("p (s j) -> p s j", j=1).broadcast_to(
                (128, SW, V))
            nc.vector.tensor_mul(
                a_t[:, :].rearrange("p (s v) -> p s v", v=V),
                e_t[:, :].rearrange("p (s v) -> p s v", v=V), rc_bc)
            return (s0, cnt, x_ts, a_t)

        def tail(st, drain=False):
            s0, cnt, x_ts, a_t = st
            n0g = s0 * npp * TILE_N
            for j in range(cnt):
                x_t = x_ts[j]
                o_t = out_pool.tile([128, npp * D], BF16)
                ps = psum_pool.tile([128, npp * D], F32)
                for m in range(npp):
                    s = j * npp + m
                    diag = diag_pool.tile([128, V * 128], BF16)
                    if diag_pat[m][0] == "f":
                        # all 3 diag blocks in ONE DVE op: ident3 * a broadcast
                        a_bc = a_t[:, s * V:(s + 1) * V].rearrange(
                            "p (v j) -> p v j", j=1).broadcast_to((128, V, 128))
                        nc.vector.tensor_mul(
                            diag[:, :].rearrange("p (v j) -> p v j", v=V),
                            ident3[:, :].rearrange("p (v j) -> p v j", v=V),
                            a_bc)
                    else:
                        for v in range(V):
                            sl = diag[:, v * 128:(v + 1) * 128]
                            ev = a_t[:, s * V + v:s * V + v + 1]
                            deng = diag_pat[m][v]
                            if drain:
                                deng = "d"
                            if deng == "a":
                                nc.scalar.activation(
                                    sl, ident[:, :], AF.Copy, bias=0.0,
                                    scale=ev)
                            else:
                                nc.vector.tensor_scalar_mul(
                                    sl, ident[:, :], ev)

                    for v in range(V):
                        nc.tensor.matmul(
                            ps[:, m * D:(m + 1) * D],
                            diag[:, v * 128:(v + 1) * 128],
                            x_t[:, (v * npp + m) * D:(v * npp + m + 1) * D],
                            start=(v == 0),
                            stop=(v == V - 1),
                        )

                # one batched PSUM->SBUF cast per supertile (1/den already
                # folded into the diag weights)
                nc.scalar.activation(o_t[:, :], ps[:, :], AF.Copy)
                n0 = n0g + j * npp * TILE_N
                stq.append((o_t, n0))

        stq = []

        def do_store(o_t, n0):
            dstO = out[n0:n0 + npp * TILE_N, :].rearrange(
                "(p m) d -> p m d", m=npp)
            store.dma_start(
                dstO, o_t[:, :].rearrange("p (m d) -> p m d", m=npp))

        taper_st = taper  # head/tail supertiles emitted as single-st groups
        groups = []
        i = 0
        while i < NST:
            if i < front_taper or i >= NST - taper_st:
                groups.append((i, 1))
                i += 1
            else:
                groups.append((i, grp))
                i += grp
        drain_start = NST - tail_norm_dve * grp
        pend_mid, pend_tail = [], []
        for (s0, cnt) in groups:
            pend_mid.append(head(s0, cnt))
            if len(pend_mid) > 1:
                pend_tail.append(mid(pend_mid.pop(0)))
            while len(pend_tail) > max(depth - 1, 1):
                st = pend_tail.pop(0)
                tail(st, drain=st[0] >= drain_start)
            while len(stq) > store_lag * grp:
                do_store(*stq.pop(0))
        while pend_mid:
            pend_tail.append(mid(pend_mid.pop(0)))
        while pend_tail:
            st = pend_tail.pop(0)
            tail(st, drain=st[0] >= drain_start)
        while stq:
            do_store(*stq.pop(0))

    nc.compile()
    return nc


def _make_in_maps(xs, alpha_w, alpha_b):
    xs = np.asarray(xs, dtype=np.float32).astype(ml_dtypes.bfloat16)
    aw = (np.asarray(alpha_w, dtype=np.float32).reshape(1, D)
          .astype(ml_dtypes.bfloat16))
    awb = np.ascontiguousarray(np.broadcast_to(aw, (128, D)))
    abc = np.full((128, 1), np.asarray(alpha_b, np.float32).reshape(()),
                  dtype=np.float32)
    idn = np.eye(128, dtype=ml_dtypes.bfloat16)
    idn3 = np.ascontiguousarray(np.concatenate([idn] * V, axis=1))
    in_maps = []
    for i in range(NCORES):
        in_maps.append({
            "xs": np.ascontiguousarray(xs[:, i * NL:(i + 1) * NL, :]),
            "w_bcast": awb,
            "b_colv": abc,
            "identity": idn,
            "identity3": idn3,
        })
    return in_maps


def run(xs, alpha_w, alpha_b, trace=False):
    nc = build_nc2()
    in_maps = _make_in_maps(xs, alpha_w, alpha_b)
    res = run_bass_kernel_spmd(nc, in_maps, list(range(NCORES)), trace=trace)
    out = np.concatenate(
        [np.asarray(res.results[i]["out"]) for i in range(NCORES)], axis=0
    ).astype(np.float32)
    return out, res


def kernel(xs, alpha_w, alpha_b):
    out, _ = run(xs, alpha_w, alpha_b, trace=False)
    return out

